# revision 1
# baseline (speedup 1.0000x reference)
"""GNN message passing (gather + scatter-add) on 8 trn2 NeuronCores.

Strategy: shard by destination node range (12500 nodes per core). Host
sorts each core's edges by destination tile (128 dst nodes per tile,
`cap` blocks of 128 edges per tile, padded). On device, per tile:
  1. indirect-DMA gather of x rows (packed hi/lo bf16, 512B each) for
     cap*128 edges in ONE instruction,
  2. one DVE compare builds the one-hot selection matrix for all blocks,
  3. cap accumulating matmuls: psum[dst,0:256] += sel^T @ msg(hi|lo),
  4. psum hi-half + lo-half -> out rows, streamed to DRAM.
No collective needed; each core owns its output slice.
"""

import os
import sys

import numpy as np
import ml_dtypes

for _p in ("/opt/trn_rl_repo",):
    if _p not in sys.path:
        sys.path.insert(0, _p)

from concourse import bass, mybir, tile, bacc  # noqa: E402
from concourse.bass_utils import run_bass_kernel_spmd  # noqa: E402

P = 128
D = 128
N_NODES = 100000
N_CORES = 8
NODES_PER_CORE = N_NODES // N_CORES  # 12500
PAD_SRC = 1 << 20  # out-of-bounds sentinel: gather skips it


def build_program(n_nodes, n_tiles, cap, num_devices):
    """One SPMD program: per-core inputs srcT/dstT select this core's edges."""
    nblk = n_tiles * cap
    nc = bacc.Bacc(
        "TRN2", target_bir_lowering=False, debug=False, num_devices=num_devices
    )
    xp = nc.dram_tensor(
        "xp", [n_nodes, 2 * D], mybir.dt.bfloat16, kind="ExternalInput"
    ).ap()
    srcT = nc.dram_tensor("srcT", [P, nblk], mybir.dt.int32, kind="ExternalInput").ap()
    dstT = nc.dram_tensor("dstT", [P, nblk], mybir.dt.int32, kind="ExternalInput").ap()
    iota = nc.dram_tensor("iota", [P, P], mybir.dt.int32, kind="ExternalInput").ap()
    out = nc.dram_tensor(
        "out", [n_tiles * P, D], mybir.dt.float32, kind="ExternalOutput"
    ).ap()

    with tile.TileContext(nc) as tc:
        with tc.tile_pool(name="sb", bufs=1) as pool, tc.tile_pool(
            name="ps", bufs=1, space="PSUM"
        ) as psp:
            srcs = pool.tile([P, nblk], mybir.dt.int32)
            dsts = pool.tile([P, nblk], mybir.dt.int32)
            iot = pool.tile([P, P], mybir.dt.int32)
            nc.sync.dma_start(out=srcs[:], in_=srcT[:])
            nc.sync.dma_start(out=dsts[:], in_=dstT[:])
            nc.sync.dma_start(out=iot[:], in_=iota[:])

            nbuf = 2
            msg = [pool.tile([P, cap * 2 * D], mybir.dt.bfloat16, name=f"msg{i}") for i in range(nbuf)]
            sel = [pool.tile([P, cap * P], mybir.dt.bfloat16, name=f"sel{i}") for i in range(nbuf)]
            stg = [pool.tile([P, D], mybir.dt.float32, name=f"stg{i}") for i in range(nbuf)]
            pst = [
                psp.tile([P, D], dtype=mybir.dt.float32, space="PSUM", name=f"pst{i}")
                for i in range(nbuf)
            ]
            # OOB-skipped (padding) gather slots keep stale SBUF contents on
            # hw; zero them once so pad rows are finite (sel column is 0).
            for m in msg:
                nc.vector.memset(m[:], 0)

            for t in range(n_tiles):
                k = t % nbuf
                mg, sl, sg, ps = msg[k], sel[k], stg[k], pst[k]
                c0 = t * cap
                # One gather per 128-edge block: the HW vector-indirect DMA
                # honors one offset per partition (multi-column offset
                # tables silently degrade to first-column + contiguous).
                for b in range(cap):
                    nc.gpsimd.indirect_dma_start(
                        out=mg[:, b * 2 * D : (b + 1) * 2 * D],
                        out_offset=None,
                        in_=xp[:],
                        in_offset=bass.IndirectOffsetOnAxis(
                            ap=srcs[:, c0 + b : c0 + b + 1], axis=0
                        ),
                        bounds_check=n_nodes - 1,
                        oob_is_err=False,
                    )
                nc.vector.tensor_tensor(
                    out=sl[:],
                    in0=dsts[:, c0 : c0 + cap][:, :, None].to_broadcast([P, cap, P]),
                    in1=iot[:, None, :].to_broadcast([P, cap, P]),
                    op=mybir.AluOpType.is_equal,
                )
                for b in range(cap):
                    # hi and lo halves accumulate into the SAME psum columns,
                    # sharing the stationary sel matrix.
                    nc.tensor.matmul(
                        out=ps[:],
                        lhsT=sl[:, b * P : (b + 1) * P],
                        rhs=mg[:, b * 2 * D : b * 2 * D + D],
                        start=(b == 0),
                        stop=False,
                    )
                    nc.tensor.matmul(
                        out=ps[:],
                        lhsT=sl[:, b * P : (b + 1) * P],
                        rhs=mg[:, b * 2 * D + D : (b + 1) * 2 * D],
                        start=False,
                        stop=(b == cap - 1),
                    )
                nc.scalar.copy(sg[:], ps[:])
                nc.sync.dma_start(out=out[t * P : (t + 1) * P, :], in_=sg[:])
    nc.compile()
    return nc


def pack_x(x):
    """[N, D] f32 -> [N, 2D] bf16 rows: hi | lo with x ~= hi + lo."""
    x = np.asarray(x, dtype=np.float32)
    hi = x.astype(ml_dtypes.bfloat16)
    lo = (x - hi.astype(np.float32)).astype(ml_dtypes.bfloat16)
    return np.ascontiguousarray(np.concatenate([hi, lo], axis=1))


def prep_core(src, dst, core, cap, n_tiles, nodes_per_core):
    """Bin one core's edges by dst tile into [P, n_tiles*cap] index planes."""
    lo = core * nodes_per_core
    m = (dst >= lo) & (dst < lo + nodes_per_core)
    es = src[m].astype(np.int64)
    ed = (dst[m] - lo).astype(np.int64)
    tid = ed >> 7
    rel = ed & 127
    order = np.argsort(tid, kind="stable")
    es, rel, tid = es[order], rel[order], tid[order]
    counts = np.bincount(tid, minlength=n_tiles)
    if counts.max() > cap * P:
        raise ValueError(f"cap {cap} too small for tile count {counts.max()}")
    starts = np.zeros(n_tiles, dtype=np.int64)
    starts[1:] = np.cumsum(counts)[:-1]
    pos = np.arange(len(es)) - starts[tid]
    slot = tid * (cap * P) + pos
    total = n_tiles * cap * P
    srcflat = np.full(total, PAD_SRC, np.int32)
    dstflat = np.full(total, -1, np.int32)
    srcflat[slot] = es
    dstflat[slot] = rel
    srcT = np.ascontiguousarray(srcflat.reshape(n_tiles * cap, P).T)
    dstT = np.ascontiguousarray(dstflat.reshape(n_tiles * cap, P).T)
    return srcT, dstT


def max_cap(dst, n_cores, nodes_per_core):
    """Blocks-of-128 needed for the fullest (core, dst-tile) bin.

    Tiles are per-core: core c, tile t covers [c*npc + t*128, ...), and
    npc is not a multiple of 128, so bin per-core rather than globally.
    """
    dst = np.asarray(dst, dtype=np.int64)
    n_tiles = -(-nodes_per_core // P)
    core = dst // nodes_per_core
    tid = core * n_tiles + ((dst - core * nodes_per_core) >> 7)
    counts = np.bincount(tid, minlength=n_cores * n_tiles)
    return int(-(-counts.max() // P))


_cache = {}


def kernel(x, edge_index):
    x = np.asarray(x, dtype=np.float32)
    edge_index = np.asarray(edge_index)
    src = edge_index[0].astype(np.int64)
    dst = edge_index[1].astype(np.int64)

    n_tiles = -(-NODES_PER_CORE // P)  # 98
    cap = max_cap(dst, N_CORES, NODES_PER_CORE)

    key = (N_NODES, n_tiles, cap)
    if key not in _cache:
        _cache[key] = build_program(N_NODES, n_tiles, cap, N_CORES)
    nc = _cache[key]

    xp = pack_x(x)
    iota = np.tile(np.arange(P, dtype=np.int32), (P, 1))
    in_maps = []
    for c in range(N_CORES):
        srcT, dstT = prep_core(src, dst, c, cap, n_tiles, NODES_PER_CORE)
        in_maps.append({"xp": xp, "srcT": srcT, "dstT": dstT, "iota": iota})

    trace = bool(int(os.environ.get("KERNEL_TRACE", "0")))
    res = run_bass_kernel_spmd(
        nc, in_maps, core_ids=list(range(N_CORES)), trace=trace
    )
    if trace:
        kernel.last_results = res
    outs = [res.results[c]["out"][:NODES_PER_CORE] for c in range(N_CORES)]
    return np.ascontiguousarray(np.concatenate(outs, axis=0))



# revision 5
# speedup vs baseline: 1.2999x; 1.2999x over previous
"""GNN message passing (gather + scatter-add) on 8 trn2 NeuronCores.

Strategy: shard by destination node range (12500 nodes per core). Host
sorts each core's edges by (dst tile, src chunk); src indices are made
chunk-relative so they fit dma_gather's int16 index planes (4 chunks of
25000 x-rows each). On device, tiles are processed in groups of 7:
  1. FOUR dma_gather instructions per group (one per src chunk), each
     fetching thousands of 256B bf16 x-rows in ONE Q7 dispatch — this
     amortizes the ~1us SWDGE fixed cost that dominated the per-block
     indirect-DMA baseline,
  2. one DVE compare builds the one-hot selection matrix for the group,
  3. per dst tile, accumulating matmuls psum[dst,:] += sel^T @ msg,
  4. psum -> SBUF copy, streamed to DRAM.
Pad slots carry src index 0 and dst -1: the gather fetches a harmless
row, the sel column stays all-zero. No collective needed; each core owns
its output slice.
"""

import os
import sys

import numpy as np
import ml_dtypes

for _p in ("/opt/trn_rl_repo",):
    if _p not in sys.path:
        sys.path.insert(0, _p)

from concourse import bass, mybir, tile, bacc  # noqa: E402
from concourse.bass_utils import run_bass_kernel_spmd  # noqa: E402

P = 128
D = 128
N_NODES = 100000
N_CORES = 8
NODES_PER_CORE = N_NODES // N_CORES  # 12500
N_CHUNKS = 4
CHUNK = 25000  # int16 gather indices must stay < 32768
GT = 7  # dst tiles per pipeline group


def build_program(n_nodes, chunk, n_chunks, n_tiles, gt, caps, num_devices):
    """One SPMD program; per-core inputs idxT/dstT select this core's edges.

    caps[k] = 128-slot blocks per (tile, chunk-k) segment, uniform across
    tiles and cores so all 8 cores share the program.
    """
    assert n_tiles % gt == 0
    n_groups = n_tiles // gt
    spt = sum(caps) * P  # slots per tile
    tot = n_tiles * spt  # total slots per core
    nmm = tot // P  # total 128-edge blocks (= matmuls)
    gblk = gt * sum(caps)  # blocks per group
    gslot = gblk * P  # slots per group
    # block offset of chunk region k inside a group
    roff = [gt * sum(caps[:k]) for k in range(n_chunks)]

    nc = bacc.Bacc(
        "TRN2", target_bir_lowering=False, debug=False, num_devices=num_devices
    )
    xp = nc.dram_tensor(
        "xp", [n_nodes, D], mybir.dt.bfloat16, kind="ExternalInput"
    ).ap()
    idxT = nc.dram_tensor(
        "idxT", [P, tot // 16], mybir.dt.int16, kind="ExternalInput"
    ).ap()
    dstT = nc.dram_tensor("dstT", [P, nmm], mybir.dt.int16, kind="ExternalInput").ap()
    iota = nc.dram_tensor("iota", [P, P], mybir.dt.int16, kind="ExternalInput").ap()
    out = nc.dram_tensor(
        "out", [n_tiles * P, D], mybir.dt.float32, kind="ExternalOutput"
    ).ap()

    with tile.TileContext(nc) as tc:
        with tc.tile_pool(name="sb", bufs=1) as pool, tc.tile_pool(
            name="ps", bufs=1, space="PSUM"
        ) as psp:
            idx_sb = pool.tile([P, tot // 16], mybir.dt.int16)
            dst_sb = pool.tile([P, nmm], mybir.dt.int16)
            iot = pool.tile([P, P], mybir.dt.int16)
            nc.sync.dma_start(out=idx_sb[:], in_=idxT[:])
            nc.sync.dma_start(out=dst_sb[:], in_=dstT[:])
            nc.sync.dma_start(out=iot[:], in_=iota[:])

            nbuf = 2
            msg = [
                pool.tile([P, gblk, D], mybir.dt.bfloat16, name=f"msg{i}")
                for i in range(nbuf)
            ]
            sel = [
                pool.tile([P, gblk * P], mybir.dt.bfloat16, name=f"sel{i}")
                for i in range(nbuf)
            ]
            stg = [
                pool.tile([P, D], mybir.dt.float32, name=f"stg{i}")
                for i in range(nbuf * gt)
            ]
            # one PSUM bank (2KB/partition) holds 4 dst tiles side by side
            nbank = -(-gt // 4)
            pst = [
                psp.tile(
                    [P, 4 * D], dtype=mybir.dt.float32, space="PSUM", name=f"pst{i}"
                )
                for i in range(nbuf * nbank)
            ]

            for g in range(n_groups):
                i = g % nbuf
                mg, sl = msg[i], sel[i]
                for k in range(n_chunks):
                    sk = gt * caps[k] * P
                    c0 = (g * gslot + roff[k] * P) // 16
                    nc.gpsimd.dma_gather(
                        mg[:, roff[k] : roff[k] + gt * caps[k], :],
                        xp[k * chunk : (k + 1) * chunk, :],
                        idx_sb[:, c0 : c0 + sk // 16],
                        sk,
                        sk,
                        D,
                        # >64 descriptors per engine overflows the single-
                        # packet ceiling and wedges the exec unit on HW
                        single_packet=False,
                    )
                nc.vector.tensor_tensor(
                    out=sl[:],
                    in0=dst_sb[:, g * gblk : (g + 1) * gblk][:, :, None].to_broadcast(
                        [P, gblk, P]
                    ),
                    in1=iot[:, None, :].to_broadcast([P, gblk, P]),
                    op=mybir.AluOpType.is_equal,
                )
                for t in range(gt):
                    ps = pst[i * nbank + t // 4][:, (t % 4) * D : (t % 4 + 1) * D]
                    blks = [
                        roff[k] + t * caps[k] + b
                        for k in range(n_chunks)
                        for b in range(caps[k])
                    ]
                    for j, blk in enumerate(blks):
                        nc.tensor.matmul(
                            out=ps,
                            lhsT=sl[:, blk * P : (blk + 1) * P],
                            rhs=mg[:, blk, :],
                            start=(j == 0),
                            stop=(j == len(blks) - 1),
                        )
                    sg = stg[i * gt + t]
                    nc.scalar.copy(sg[:], ps)
                    row = (g * gt + t) * P
                    nc.sync.dma_start(out=out[row : row + P, :], in_=sg[:])
    nc.compile()
    return nc


def compute_caps(src, dst, n_cores, nodes_per_core, chunk, n_chunks, n_tiles):
    """caps[k]: blocks needed for the fullest (core, tile, chunk-k) bin."""
    src = np.asarray(src, dtype=np.int64)
    dst = np.asarray(dst, dtype=np.int64)
    core = dst // nodes_per_core
    t = (dst - core * nodes_per_core) >> 7
    k = src // chunk
    gid = (core * n_tiles + t) * n_chunks + k
    counts = np.bincount(gid, minlength=n_cores * n_tiles * n_chunks)
    per_k = counts.reshape(-1, n_chunks).max(axis=0)
    return tuple(int(-(-c // P)) for c in per_k)


def prep_core(src, dst, core, caps, n_tiles, gt, nodes_per_core, chunk, n_chunks):
    """Bin one core's edges into slot planes.

    Slot order: (group, chunk, tile-in-group, position); each (tile, chunk)
    segment is caps[k]*128 slots. Returns idxT [128, tot/16] int16 (gather
    plane: slot s -> partition s%16 replicated over the 8 Q7 groups, col
    s/16) and dstT [128, tot/128] int16 (slot s -> partition s%128, block
    s/128; -1 on pad slots).
    """
    spt = sum(caps) * P
    tot = n_tiles * spt
    gslot = gt * spt
    soff = [gt * P * sum(caps[:k]) for k in range(n_chunks)]

    lo = core * nodes_per_core
    m = (dst >= lo) & (dst < lo + nodes_per_core)
    es = src[m].astype(np.int64)
    ed = (dst[m] - lo).astype(np.int64)
    t = ed >> 7
    rel = ed & 127
    k = es // chunk
    srel = (es - k * chunk).astype(np.int64)

    bin_id = t * n_chunks + k
    order = np.argsort(bin_id, kind="stable")
    bid = bin_id[order]
    t, k, rel, srel = t[order], k[order], rel[order], srel[order]
    counts = np.bincount(bid, minlength=n_tiles * n_chunks)
    starts = np.zeros(n_tiles * n_chunks, dtype=np.int64)
    starts[1:] = np.cumsum(counts)[:-1]
    pos = np.arange(len(bid)) - starts[bid]
    sbase = (t // gt) * gslot + np.array(soff)[k] + (t % gt) * (np.array(caps)[k] * P)
    slot = sbase + pos

    srcflat = np.zeros(tot, np.int16)  # pad slots gather row 0 of their chunk
    dstflat = np.full(tot, -1, np.int16)
    srcflat[slot] = srel
    dstflat[slot] = rel
    idx16 = srcflat.reshape(tot // 16, 16).T  # [16, tot/16]
    idxT = np.ascontiguousarray(np.tile(idx16, (8, 1)))
    dstT = np.ascontiguousarray(dstflat.reshape(tot // P, P).T)
    return idxT, dstT


_cache = {}


def kernel(x, edge_index):
    x = np.asarray(x, dtype=np.float32)
    edge_index = np.asarray(edge_index)
    src = edge_index[0].astype(np.int64)
    dst = edge_index[1].astype(np.int64)

    n_tiles = -(-NODES_PER_CORE // P)  # 98
    caps = compute_caps(src, dst, N_CORES, NODES_PER_CORE, CHUNK, N_CHUNKS, n_tiles)

    key = (N_NODES, n_tiles, caps)
    if key not in _cache:
        _cache[key] = build_program(
            N_NODES, CHUNK, N_CHUNKS, n_tiles, GT, caps, N_CORES
        )
    nc = _cache[key]

    xp = np.ascontiguousarray(x.astype(ml_dtypes.bfloat16))
    iota = np.tile(np.arange(P, dtype=np.int16), (P, 1))
    in_maps = []
    for c in range(N_CORES):
        idxT, dstT = prep_core(
            src, dst, c, caps, n_tiles, GT, NODES_PER_CORE, CHUNK, N_CHUNKS
        )
        in_maps.append({"xp": xp, "idxT": idxT, "dstT": dstT, "iota": iota})

    trace = bool(int(os.environ.get("KERNEL_TRACE", "0")))
    res = run_bass_kernel_spmd(
        nc, in_maps, core_ids=list(range(N_CORES)), trace=trace
    )
    if trace:
        kernel.last_results = res
    outs = [res.results[c]["out"][:NODES_PER_CORE] for c in range(N_CORES)]
    return np.ascontiguousarray(np.concatenate(outs, axis=0))
